# revision 27
# baseline (speedup 1.0000x reference)
"""Trainium2 Bass kernel for nn_NeuralMemory (Titans-style neural memory).

Sharding: (batch*heads) = 16 pairs over 8 cores; core i handles batch i//2,
heads (0,1) or (2,3). Each core computes a partial output (its 2 heads'
contribution through W_o); host sums the two partials per batch.

Per-core algorithm (all compute on device):
  rmsnorm scale -> t = x*scale -> tT via PE transposes
  projections kT/vT/qT + lr/gate/mom/decay from tT (f32r matmuls)
  chunk grads via n-layout bf16 matmuls with lr/(2/d)/sign folded into drains
  momentum+decay double recurrence via DVE tensor_tensor_scan in a
  (128, j*65) slot layout (slot 0 = initial weights, gate 0 resets state)
  retrieval from scanned weights (strided APs), multihead rmsnorm + gate,
  partial out = og @ (gamma-folded W_o)
"""
import numpy as np

N = 4096
DIM = 512
D = 128
C = 64
NCH = 64          # chunks
NT = N // 128     # 32 n-tiles
KT = DIM // 128   # 4 dim tiles
EPS = 1e-6
SLOT = NCH + 1    # 65 slots per j-group
JB = 32           # j-block for scans
SCANW = 128 * SLOT  # 8320

_CACHE = {}


def _build():
    import concourse.bass as bass
    import concourse.mybir as mybir
    import concourse.tile as tile
    from concourse import bacc
    from concourse.masks import make_identity

    dt = mybir.dt
    AF = mybir.ActivationFunctionType
    ALU = mybir.AluOpType

    nc = bacc.Bacc('TRN2', target_bir_lowering=False)

    x_d = nc.dram_tensor("x", (N, DIM), dt.float32, kind="ExternalInput")
    wbig_d = nc.dram_tensor("wbig", (DIM, 768), dt.float32, kind="ExternalInput")
    wsm_d = nc.dram_tensor("wsm", (DIM, 8), dt.float32, kind="ExternalInput")
    w1_d = nc.dram_tensor("w1", (D, D), dt.float32, kind="ExternalInput")
    w2_d = nc.dram_tensor("w2", (D, D), dt.float32, kind="ExternalInput")
    w2t_d = nc.dram_tensor("w2t", (D, D), dt.float32, kind="ExternalInput")
    wo_d = nc.dram_tensor("wo", (2 * D, DIM), dt.float32, kind="ExternalInput")
    out_d = nc.dram_tensor("OUT", (N, DIM), dt.float32, kind="ExternalOutput")

    with tile.TileContext(nc) as tc:
        with tc.tile_pool(name="const", bufs=1) as const, \
             tc.tile_pool(name="persist", bufs=1) as persist, \
             tc.tile_pool(name="dram", bufs=1, space="DRAM") as dram:

            # ---------------- phase 0: weights + identities ----------------
            id_r = const.tile([D, D], dt.float32r)
            id_b = const.tile([D, D], dt.bfloat16)
            idf = const.tile([D, D], dt.float32)
            make_identity(nc, idf[:])
            eps_col = const.tile([D, 1], dt.float32)
            nc.vector.memset(eps_col[:], EPS)
            nc.vector.tensor_copy(id_r[:], idf[:])
            nc.vector.tensor_copy(id_b[:], idf[:])

            wbig_f = const.tile([D, KT, 768], dt.float32)
            nc.sync.dma_start(wbig_f[:], wbig_d[:, :].rearrange("(k p) c -> p k c", p=D))
            wbig = const.tile([D, KT, 768], dt.float32r)
            nc.vector.tensor_copy(wbig[:], wbig_f[:])

            wsm_f = const.tile([D, KT, 8], dt.float32)
            nc.sync.dma_start(wsm_f[:], wsm_d[:, :].rearrange("(k p) c -> p k c", p=D))
            wsm = const.tile([D, KT, 8], dt.float32r)
            nc.vector.tensor_copy(wsm[:], wsm_f[:])

            w1f = const.tile([D, D], dt.float32)
            w2f = const.tile([D, D], dt.float32)
            w2tf = const.tile([D, D], dt.float32)
            nc.sync.dma_start(w1f[:], w1_d[:, :])
            nc.sync.dma_start(w2f[:], w2_d[:, :])
            nc.sync.dma_start(w2tf[:], w2t_d[:, :])
            w1b = const.tile([D, D], dt.bfloat16)
            w2b = const.tile([D, D], dt.bfloat16)
            w2tb = const.tile([D, D], dt.bfloat16)
            nc.vector.tensor_copy(w1b[:], w1f[:])
            nc.vector.tensor_copy(w2b[:], w2f[:])
            nc.vector.tensor_copy(w2tb[:], w2tf[:])

            wo_f = const.tile([D, 2, DIM], dt.float32)
            nc.sync.dma_start(wo_f[:], wo_d[:, :].rearrange("(h p) c -> p h c", p=D))
            wo = const.tile([D, 2, DIM], dt.bfloat16)
            nc.vector.tensor_copy(wo[:], wo_f[:])

            # ---------------- phase 1: rmsnorm + transpose ----------------
            # tT: 4 tiles (128, 4096) f32r
            tT, tT_free = tc.tile([D, KT, N], dt.float32r, name="tT")
            with tc.tile_pool(name="p1", bufs=3) as p1, \
                 tc.tile_pool(name="p1ps", bufs=3, space="PSUM") as p1ps:
                for i in range(NT):
                    xt = p1.tile([D, DIM], dt.float32, tag="xt")
                    nc.sync.dma_start(xt[:], x_d[i * 128:(i + 1) * 128, :])
                    sqd = p1.tile([D, DIM], dt.float32, tag="sqd")
                    ssq = p1.tile([D, 1], dt.float32, tag="ssq")
                    nc.scalar.activation(sqd[:], xt[:], AF.Square, accum_out=ssq[:])
                    rt = p1.tile([D, 1], dt.float32, tag="rt")
                    nc.scalar.activation(rt[:], ssq[:], AF.Sqrt,
                                         bias=eps_col[:], scale=1.0 / DIM)
                    sc = p1.tile([D, 1], dt.float32, tag="sc")
                    nc.vector.reciprocal(sc[:], rt[:])
                    tt = p1.tile([D, DIM], dt.float32r, tag="tt")
                    nc.vector.tensor_scalar_mul(tt[:], xt[:], sc[:])
                    for k in range(KT):
                        pst = p1ps.tile([D, D], dt.float32r, tag="pst")
                        nc.tensor.transpose(pst[:], tt[:, k * 128:(k + 1) * 128], id_r[:])
                        nc.scalar.copy(tT[:, k, i * 128:(i + 1) * 128], pst[:])

            # ---------------- phase 2: projections ----------------
            # per head: kT/qT bf16, vT f32 (128, 4096); order in wbig: h*384 + {k:0, v:128, q:256}
            kT = [persist.tile([D, N], dt.bfloat16, tag=f"kT{h}", name=f"kT{h}") for h in range(2)]
            vT = [persist.tile([D, N], dt.bfloat16, tag=f"vT{h}", name=f"vT{h}") for h in range(2)]
            qT = [persist.tile([D, N], dt.bfloat16, tag=f"qT{h}", name=f"qT{h}") for h in range(2)]
            lrgT, lrgT_free = tc.tile([4, N], dt.float32, name="lrgT")  # rows lr0,lr1,g0,g1
            md = persist.tile([4, NCH], dt.float32)      # rows mom0,mom1,dec0,dec1

            with tc.tile_pool(name="p2ps", bufs=4, space="PSUM") as p2ps, \
                 tc.tile_pool(name="p2", bufs=2) as p2:
                for h in range(2):
                    for pi, (dst, cof) in enumerate([(kT[h], 0), (vT[h], 128), (qT[h], 256)]):
                        for j in range(8):
                            ps = p2ps.tile([D, 512], dt.float32, tag="proj")
                            for k in range(KT):
                                nc.tensor.matmul(
                                    ps[:], wbig[:, k, h * 384 + cof: h * 384 + cof + 128],
                                    tT[:, k, j * 512:(j + 1) * 512],
                                    start=(k == 0), stop=(k == KT - 1))
                            nc.scalar.copy(dst[:, j * 512:(j + 1) * 512], ps[:])
                # lr/gate rows
                for j in range(8):
                    ps = p2ps.tile([4, 512], dt.float32, tag="lrg", bufs=2)
                    for k in range(KT):
                        nc.tensor.matmul(ps[:], wsm[:, k, 0:4],
                                         tT[:, k, j * 512:(j + 1) * 512],
                                         start=(k == 0), stop=(k == KT - 1))
                    nc.scalar.activation(lrgT[:, j * 512:(j + 1) * 512], ps[:], AF.Sigmoid)
                # chunk means -> mom/dec rows
                cmT = p2.tile([D, KT, NCH], dt.float32r, tag="cmT")
                with nc.allow_low_precision("f32r is full fp32 storage"):
                    for k in range(KT):
                        nc.vector.tensor_reduce(
                            cmT[:, k, :], tT[:, k, :].rearrange("p (n c) -> p n c", c=C),
                            axis=mybir.AxisListType.X, op=ALU.add)
                psm = p2ps.tile([4, NCH], dt.float32, tag="md", bufs=1)
                for k in range(KT):
                    nc.tensor.matmul(psm[:], wsm[:, k, 4:8], cmT[:, k, :],
                                     start=(k == 0), stop=(k == KT - 1))
                nc.scalar.activation(md[:], psm[:], AF.Sigmoid, scale=1.0 / C)

            # lr/gate column tiles (128, 32) per head + scaled variants
            lrneg = [persist.tile([D, NT], dt.float32, tag=f"lrneg{h}", name=f"lrneg{h}") for h in range(2)]
            gcol = [persist.tile([D, NT], dt.float32, tag=f"gcol{h}", name=f"gcol{h}") for h in range(2)]
            with tc.tile_pool(name="pcols", bufs=1) as pcols:
                lrg_d = dram.tile([4, N], dt.float32, tag="lrg_d")
                nc.sync.dma_start(lrg_d[:, :], lrgT[:])
                for h in range(2):
                    lc = pcols.tile([D, NT], dt.float32, tag="lc")
                    nc.sync.dma_start(
                        lc[:], lrg_d[h:h + 1, :].rearrange("o (i p) -> (o p) i", p=128))
                    nc.scalar.activation(lrneg[h][:], lc[:], AF.Copy, scale=-2.0 / D)
                    nc.sync.dma_start(
                        gcol[h][:], lrg_d[2 + h:3 + h, :].rearrange("o (i p) -> (o p) i", p=128))
            lrgT_free()
            tT_free()

            # gate pattern rows (built per head inside the head loop)
            grow_d = dram.tile([2, 2, SLOT], dt.float32, tag="grow_d")
            # md rows + (1-decay) rows bounced to DRAM for partition-free access
            md_d = dram.tile([4, NCH], dt.float32, tag="md_d")
            ndg_d = dram.tile([4, NCH], dt.float32, tag="ndg_d")
            with tc.tile_pool(name="pmd", bufs=1) as pmd:
                ndg = pmd.tile([4, NCH], dt.float32, tag="ndg")
                nc.vector.tensor_scalar(ndg[:], md[:], -1.0, 1.0,
                                        ALU.mult, ALU.add)
                nc.sync.dma_start(md_d[:, :], md[:])
                nc.sync.dma_start(ndg_d[:, :], ndg[:])

            og = [persist.tile([D, NT * D], dt.bfloat16, tag=f"og{h}", name=f"og{h}") for h in range(2)]

            # ================ per-head store + scan + retrieve ================
            for h in range(2):
                with tc.tile_pool(name=f"ph{h}", bufs=1) as ph, \
                     tc.tile_pool(name=f"ph{h}s", bufs=3) as phs, \
                     tc.tile_pool(name=f"ph{h}ps", bufs=2, space="PSUM") as phps:
                    # --- gates for this head ---
                    gates_m = ph.tile([D, JB * SLOT], dt.float32, tag="gates_m")
                    gates_w = ph.tile([D, JB * SLOT], dt.float32, tag="gates_w")
                    grow = ph.tile([1, 2 * SLOT], dt.float32, tag="grow")
                    nc.vector.memset(grow[:], 0.0)
                    # m-gates: [0, 0, mom_2..mom_64] ; w-gates: [0, dg_1..dg_64]
                    nc.sync.dma_start(grow[0:1, 2:SLOT], md_d[h:h + 1, 1:NCH])
                    nc.sync.dma_start(grow[0:1, SLOT + 1:2 * SLOT],
                                      ndg_d[2 + h:3 + h, 0:NCH])
                    nc.sync.dma_start(grow_d[h, 0], grow[0:1, 0:SLOT])
                    nc.sync.dma_start(grow_d[h, 1], grow[0:1, SLOT:2 * SLOT])
                    src_m = bass.AP(tensor=grow_d.tensor, offset=grow_d[h, 0].offset,
                                    ap=[[0, D], [0, JB], [1, SLOT]])
                    src_w = bass.AP(tensor=grow_d.tensor, offset=grow_d[h, 1].offset,
                                    ap=[[0, D], [0, JB], [1, SLOT]])
                    nc.sync.dma_start(gates_m[:], src_m)
                    nc.sync.dma_start(gates_w[:], src_w)
                    # --- 3a: HT -> AT ---
                    AT = ph.tile([D, N], dt.bfloat16, tag="AT")
                    for j in range(8):
                        ps = phps.tile([D, 512], dt.float32, tag="big")
                        nc.tensor.matmul(ps[:], w1b[:], kT[h][:, j * 512:(j + 1) * 512],
                                         start=True, stop=True)
                        nc.scalar.activation(AT[:, j * 512:(j + 1) * 512], ps[:], AF.Silu)
                    # --- 3b: PT -> dPTu ---
                    dPTu = ph.tile([D, N], dt.bfloat16, tag="dPTu")
                    for j in range(8):
                        ps = phps.tile([D, 512], dt.float32, tag="big")
                        nc.tensor.matmul(ps[:], w2b[:], AT[:, j * 512:(j + 1) * 512],
                                         start=True, stop=True)
                        nc.vector.tensor_sub(dPTu[:, j * 512:(j + 1) * 512], ps[:],
                                             vT[h][:, j * 512:(j + 1) * 512])
                    # --- scan buffers ---
                    S1 = ph.tile([D, SCANW], dt.bfloat16, tag="S1")
                    S2 = ph.tile([D, SCANW], dt.bfloat16, tag="S2")
                    # W0 into slot 0
                    nc.scalar.copy(S1[:].rearrange("p (j s) -> p j s", s=SLOT)[:, :, 0], w1b[:])
                    nc.scalar.copy(S2[:].rearrange("p (j s) -> p j s", s=SLOT)[:, :, 0], w2b[:])

                    # --- 3c pass A (Silu table): A, PmV -> s2 ---
                    for i in range(NT):
                        nsl = slice(i * 128, (i + 1) * 128)
                        psh = phps.tile([D, D], dt.float32, tag="g1")
                        nc.tensor.matmul(psh[:], kT[h][:, nsl], w1b[:], start=True, stop=True)
                        A_i = phs.tile([D, D], dt.bfloat16, tag="A_i")
                        nc.scalar.activation(A_i[:], psh[:], AF.Silu)
                        pst1 = phps.tile([D, D], dt.bfloat16, tag="g2")
                        nc.tensor.transpose(pst1[:], dPTu[:, nsl], id_b[:])
                        PmV = phs.tile([D, D], dt.bfloat16, tag="PmV")
                        nc.scalar.activation(PmV[:], pst1[:], AF.Copy,
                                             scale=lrneg[h][:, i:i + 1])
                        for c2 in range(2):
                            ch = 2 * i + c2
                            rs = slice(c2 * 64, (c2 + 1) * 64)
                            ps2 = phps.tile([D, D], dt.float32, tag="g3")
                            nc.tensor.matmul(ps2[:], A_i[rs, :], PmV[rs, :],
                                             start=True, stop=True)
                            nc.scalar.copy(
                                S2[:].rearrange("p (j s) -> p j s", s=SLOT)[:, :, ch + 1],
                                ps2[:])
                    # --- 3c pass B (Dsilu table): sp, dA, K -> s1 ---
                    for i in range(NT):
                        nsl = slice(i * 128, (i + 1) * 128)
                        psh = phps.tile([D, D], dt.float32, tag="g1")
                        nc.tensor.matmul(psh[:], kT[h][:, nsl], w1b[:], start=True, stop=True)
                        sp_i = phs.tile([D, D], dt.bfloat16, tag="sp_i")
                        nc.scalar.activation(sp_i[:], psh[:], AF.Derivative_silu)
                        psd = phps.tile([D, D], dt.float32, tag="g1")
                        nc.tensor.matmul(psd[:], dPTu[:, nsl], w2tb[:], start=True, stop=True)
                        dh_t = phs.tile([D, D], dt.bfloat16, tag="dh_t")
                        nc.vector.tensor_mul(dh_t[:], psd[:], sp_i[:])
                        dHlr = phs.tile([D, D], dt.bfloat16, tag="dHlr")
                        nc.scalar.activation(dHlr[:], dh_t[:], AF.Copy,
                                             scale=lrneg[h][:, i:i + 1])
                        pst2 = phps.tile([D, D], dt.bfloat16, tag="g2")
                        nc.tensor.transpose(pst2[:], kT[h][:, nsl], id_b[:])
                        K_n = phs.tile([D, D], dt.bfloat16, tag="K_n")
                        nc.vector.tensor_copy(K_n[:], pst2[:])
                        for c2 in range(2):
                            ch = 2 * i + c2
                            rs = slice(c2 * 64, (c2 + 1) * 64)
                            ps1 = phps.tile([D, D], dt.float32, tag="g3")
                            nc.tensor.matmul(ps1[:], K_n[rs, :], dHlr[rs, :],
                                             start=True, stop=True)
                            nc.vector.tensor_copy(
                                S1[:].rearrange("p (j s) -> p j s", s=SLOT)[:, :, ch + 1],
                                ps1[:])

                    # --- scans ---
                    W1all = ph.tile([D, SCANW], dt.bfloat16, tag="S1", name="W1all")
                    W2all = ph.tile([D, SCANW], dt.bfloat16, tag="S2", name="W2all")
                    Ms = ph.tile([D, SCANW], dt.bfloat16, tag="Ms")
                    for (S, Wall) in ((S1, W1all), (S2, W2all)):
                        for jb in range(128 // JB):
                            sl = slice(jb * JB * SLOT, (jb + 1) * JB * SLOT)
                            nc.vector.tensor_tensor_scan(
                                Ms[:, sl], gates_m[:], S[:, sl], 0.0,
                                ALU.mult, ALU.add)
                            nc.vector.tensor_tensor_scan(
                                Wall[:, sl], gates_w[:], Ms[:, sl], 0.0,
                                ALU.mult, ALU.add)

                    # --- retrieval ---
                    rsq = ph.tile([C, NCH], dt.float32, tag="rsq")
                    pred_st = ph.tile([D, NT * D], dt.bfloat16, tag="pred_st")
                    w1v = W1all[:].rearrange("p (j s) -> p s j", s=SLOT)
                    w2v = W2all[:].rearrange("p (j s) -> p s j", s=SLOT)
                    for ch in range(NCH):
                        psy = phps.tile([D, C], dt.float32, tag="big")
                        nc.tensor.matmul(psy[:], w1v[:, ch, :],
                                         qT[h][:, ch * C:(ch + 1) * C],
                                         start=True, stop=True)
                        aT = phs.tile([D, C], dt.bfloat16, tag="aT")
                        nc.scalar.activation(aT[:], psy[:], AF.Silu)
                        psp = phps.tile([C, D], dt.float32, tag="big")
                        nc.tensor.matmul(psp[:], aT[:], w2v[:, ch, :],
                                         start=True, stop=True)
                        # drain + sumsq
                        i, c2 = ch // 2, ch % 2
                        rows = slice(c2 * 64, (c2 + 1) * 64)
                        sqd = phs.tile([C, D], dt.float32, tag="sqd2")
                        nc.scalar.activation(sqd[:], psp[:], AF.Square,
                                             accum_out=rsq[:, ch:ch + 1])
                        nc.vector.tensor_copy(
                            pred_st[rows, i * 128:(i + 1) * 128], psp[:])
                    # batched rms scale * gate
                    rt2 = ph.tile([C, NCH], dt.float32, tag="rt2")
                    nc.scalar.activation(rt2[:], rsq[:], AF.Sqrt,
                                         bias=eps_col[0:C, :], scale=1.0 / D)
                    rc2 = ph.tile([C, NCH], dt.float32, tag="rc2")
                    nc.vector.reciprocal(rc2[:], rt2[:])
                    # reshape (64, 64) -> (128, 32): two DMAs (even/odd chunks)
                    gsc = ph.tile([D, NT], dt.float32, tag="gsc")
                    nc.sync.dma_start(gsc[0:C, :], rc2[:, :].rearrange("p (i a) -> p a i", a=2)[:, 0, :])
                    nc.sync.dma_start(gsc[C:D, :], rc2[:, :].rearrange("p (i a) -> p a i", a=2)[:, 1, :])
                    gsg = ph.tile([D, NT], dt.float32, tag="gsg")
                    nc.vector.tensor_mul(gsg[:], gsc[:], gcol[h][:])
                    for i in range(NT):
                        nc.vector.tensor_scalar_mul(
                            og[h][:, i * 128:(i + 1) * 128],
                            pred_st[:, i * 128:(i + 1) * 128], gsg[:, i:i + 1])

            # ---------------- phase 5: out = sum_h og_h @ Wo_h ----------------
            with tc.tile_pool(name="p5", bufs=3) as p5, \
                 tc.tile_pool(name="p5ps", bufs=3, space="PSUM") as p5ps:
                for i in range(NT):
                    pso = p5ps.tile([D, DIM], dt.float32, tag="pso")
                    for h in range(2):
                        pst = p5ps.tile([D, D], dt.bfloat16, tag="ogt")
                        nc.tensor.transpose(pst[:], og[h][:, i * 128:(i + 1) * 128], id_b[:])
                        ogT = p5.tile([D, D], dt.bfloat16, tag="ogT")
                        nc.scalar.copy(ogT[:], pst[:])
                        nc.tensor.matmul(pso[:], ogT[:], wo[:, h, :],
                                         start=(h == 0), stop=(h == 1))
                    ot = p5.tile([D, DIM], dt.float32, tag="ot")
                    nc.vector.tensor_copy(ot[:], pso[:])
                    nc.sync.dma_start(out_d[i * 128:(i + 1) * 128, :], ot[:])

    nc.compile()
    return nc


def _prep_core_inputs(inputs):
    """Host-side sharding: slice + fold small weights. Returns list of 8 in_maps."""
    seq = np.ascontiguousarray(np.asarray(inputs['seq'], dtype=np.float32))
    g_s = np.asarray(inputs['g_store'], dtype=np.float32)
    g_r = np.asarray(inputs['g_retrieve'], dtype=np.float32)
    W_q = np.asarray(inputs['W_q'], dtype=np.float32)
    W_kv = np.asarray(inputs['W_kv'], dtype=np.float32)
    W_step = np.asarray(inputs['W_step'], dtype=np.float32)
    W_mom = np.asarray(inputs['W_mom'], dtype=np.float32)
    W_decay = np.asarray(inputs['W_decay'], dtype=np.float32)
    W_gate = np.asarray(inputs['W_gate'], dtype=np.float32)
    gamma = np.asarray(inputs['gamma_mh'], dtype=np.float32)
    W_o = np.asarray(inputs['W_o'], dtype=np.float32)
    M_W1 = np.asarray(inputs['M_W1'], dtype=np.float32)
    M_W2 = np.asarray(inputs['M_W2'], dtype=np.float32)

    di = 4 * D
    Wkvg = g_s[:, None] * W_kv
    Wqg = g_r[:, None] * W_q
    in_maps = []
    for core in range(8):
        b, h0 = core // 2, (core % 2) * 2
        wbig = np.empty((DIM, 768), np.float32)
        for hh in range(2):
            hsrc = h0 + hh
            wbig[:, hh * 384 + 0:hh * 384 + 128] = Wkvg[:, hsrc * D:(hsrc + 1) * D]
            wbig[:, hh * 384 + 128:hh * 384 + 256] = Wkvg[:, di + hsrc * D:di + (hsrc + 1) * D]
            wbig[:, hh * 384 + 256:hh * 384 + 384] = Wqg[:, hsrc * D:(hsrc + 1) * D]
        wsm = np.stack([g_s * W_step[:, h0], g_s * W_step[:, h0 + 1],
                        g_r * W_gate[:, h0], g_r * W_gate[:, h0 + 1],
                        g_s * W_mom[:, h0], g_s * W_mom[:, h0 + 1],
                        g_s * W_decay[:, h0], g_s * W_decay[:, h0 + 1]], axis=1)
        wo = np.concatenate([
            (1.0 + gamma[h0, 0])[:, None] * W_o[h0 * D:(h0 + 1) * D],
            (1.0 + gamma[h0 + 1, 0])[:, None] * W_o[(h0 + 1) * D:(h0 + 2) * D]], axis=0)
        in_maps.append(dict(
            x=np.ascontiguousarray(seq[b]),
            wbig=wbig, wsm=np.ascontiguousarray(wsm),
            w1=M_W1, w2=M_W2, w2t=np.ascontiguousarray(M_W2.T),
            wo=np.ascontiguousarray(wo)))
    return in_maps


def kernel(**inputs):
    from concourse.bass_utils import run_bass_kernel_spmd
    if 'nc' not in _CACHE:
        _CACHE['nc'] = _build()
    nc = _CACHE['nc']
    in_maps = _prep_core_inputs(inputs)
    res = run_bass_kernel_spmd(nc, in_maps, core_ids=list(range(8))).results
    B = 4
    out = np.empty((B, N, DIM), np.float32)
    for b in range(B):
        out[b] = res[2 * b]['OUT'] + res[2 * b + 1]['OUT']
    return out
